# revision 13
# baseline (speedup 1.0000x reference)
"""AeroModel (gram-schmidt frame + tiny MLP) Trainium2 kernel, v2.

Self-contained: hardcodes shapes B=2097152, H=32, 8-core data-parallel sharding.
kernel(**inputs) takes full unsharded inputs, returns full [B,3] float32 output.

Math (equivalent to the reference, avoids materializing the rotation matrix):
    nv  = |v|            s1 = 1/nv
    dt  = v.w            f1 = dt*s1            (= w . v_on)
    nw2 = |w|^2 - f1^2   nw = sqrt(nw2)        (= |w_orth|)   s2 = 1/nw
    feat = [nv, f1, nw]
    y = MLP(feat)        (H=32, leaky-relu 0.01, gated 2nd layer)
    out = a*v + b*w + c*(v x w) + bias
      where b = y1*s2, a = s1*(y0 - b*f1), c = y2*s1*s2

v2 structure vs v1:
  - custom DVE ops: LEAKY_B (fused bias+leaky, one op) and GATE_LEAKY_B
    (fused bias+leaky+gating mul, one op) collapse the MLP evacuations.
  - planar bf16 geometry: v/w deinterleaved once on gpsimd, all elementwise
    at DVE 2x bf16 rate; 3 reductions done with two 3-plane-wide adds.
  - MG=512 big-tiles with double-buffered geo pool so big-tile N+1's
    DMA/front-end overlaps N's MLP/backend.
"""
import os
import numpy as np
from contextlib import ExitStack

import concourse.bass as bass
import concourse.tile as tile
from concourse import bacc, mybir
from concourse.bass_utils import run_bass_kernel_spmd
from concourse.masks import make_identity

from concourse.dve_ops import DveOp, OPS, CUSTOM_DVE_SPECS, _SUB_OPCODE_FOR_NAME
from concourse.dve_spec import Spec, Src0, Src1, C0, C1, maxx, sq, lower
from concourse.dve_uop import DveOpSpec

AF = mybir.ActivationFunctionType
ALU = mybir.AluOpType
FP32 = mybir.dt.float32
BF16 = mybir.dt.bfloat16

B = 2097152
NCORES = 8
BC = B // NCORES          # rows per core
P = 128
MG = int(os.environ.get("K_MG", "512"))   # rows per partition per big-tile
ROWS_BIG = P * MG
NM2 = 1024                # MLP macro columns (32 m-values x 32 hidden)
SLOPE = 0.01
LOOP_MODE = os.environ.get("K_LOOP", "plain")
GEO_BUFS = int(os.environ.get("K_GEOBUFS", "2"))
MLP_BUFS = int(os.environ.get("K_MLPBUFS", "3"))
PSM_BUFS = int(os.environ.get("K_PSM", "2"))
PST_BUFS = int(os.environ.get("K_PST", "2"))
PSY_BUFS = int(os.environ.get("K_PSY", "2"))
# engine split knobs: ACT share (columns out of 1024) for each MLP layer
H1_ACT = int(os.environ.get("K_H1ACT", "1024"))  # h1: rest on DVE custom
G_ACT = int(os.environ.get("K_GACT", "0"))       # gate: ACT prelu + DVE mul share
OBY_POOL = int(os.environ.get("K_OBYPOOL", "2")) # oby STT planes on Pool
IL = int(os.environ.get("K_IL", "2"))            # macro interleave width
Y1_ACT = int(os.environ.get("K_Y1ACT", "512"))   # y1: rest on DVE custom
PSUM_BF16 = int(os.environ.get("K_PSUMBF16", "0"))


# ---------------- custom DVE ops ----------------
def _register_op(name, spec, subdim=False):
    if name in _SUB_OPCODE_FOR_NAME:
        for op in OPS:
            if op.name == name:
                return op
    shas = {}
    for ver in ("v3", "v4"):
        uops = lower(spec, ver=ver)
        shas[ver] = DveOpSpec(name=name, opcode=1, uops=uops).sha(ver)
    op = DveOp(name, spec, subdim=subdim, uops_sha=shas)
    OPS.append(op)
    CUSTOM_DVE_SPECS[op.name] = op.spec
    _SUB_OPCODE_FOR_NAME[op.name] = max(_SUB_OPCODE_FOR_NAME.values()) + 1
    assert _SUB_OPCODE_FOR_NAME[op.name] < 0x20
    return op


_t = Src0 + C0
LEAKY_B = _register_op(
    "LEAKY_B",
    Spec(body=maxx(_t, _t * C1),
         reference=lambda in0, in1, s0, s1, imm2: np.maximum(
             in0.astype(np.float32) + s0, (in0.astype(np.float32) + s0) * s1)),
)
GATE_LEAKY_B = _register_op(
    "GATE_LEAKY_B",
    Spec(body=maxx(_t, _t * C1) * Src1,
         reference=lambda in0, in1, s0, s1, imm2: np.maximum(
             in0.astype(np.float32) + s0,
             (in0.astype(np.float32) + s0) * s1) * in1),
)
# nw2 = max(nwsq - f1^2, eps): guards Sqrt against bf16-rounding negatives
SUBSQ_CLAMP = _register_op(
    "SUBSQ_CLAMP",
    Spec(body=maxx(Src1 - sq(Src0), C1),
         reference=lambda in0, in1, s0, s1, imm2: np.maximum(
             in1.astype(np.float32) - in0.astype(np.float32) ** 2, s1)),
)


def _body(ctx, tc, nbig, v_d, w_d, wts, out_d, repeat=1, phase='full'):
    nc = tc.nc
    W1_d, b1_d, W2_d, b2_d, Wd1_d, bd1_d, Wd2_d, bd2_d, bias_d = wts
    MMDT = BF16 if PSUM_BF16 else FP32

    singles = ctx.enter_context(tc.tile_pool(name="singles", bufs=1))
    geo = ctx.enter_context(tc.tile_pool(name="geo", bufs=GEO_BUFS))
    mlp = ctx.enter_context(tc.tile_pool(name="mlp", bufs=MLP_BUFS))
    psT = ctx.enter_context(tc.tile_pool(name="psT", bufs=PST_BUFS, space="PSUM"))
    psM = ctx.enter_context(tc.tile_pool(name="psM", bufs=PSM_BUFS, space="PSUM"))
    psY = ctx.enter_context(tc.tile_pool(name="psY", bufs=PSY_BUFS, space="PSUM"))

    # ---------------- one-time prep ----------------
    ident = singles.tile([P, P], FP32)
    make_identity(nc, ident)
    identb = singles.tile([P, P], BF16, tag="identb")
    nc.vector.tensor_copy(out=identb, in_=ident)

    # block-diagonal lhsT weights (4 groups); transposes done on PE
    w1q = []
    for jj in range(8):
        t = singles.tile([96, P], FP32, tag=f"w1q{jj}")
        nc.vector.memset(t, 0.0)
        w1q.append(t)
    w2bd = singles.tile([P, P], FP32)        # lhsT[32g+c, 32g+i] = W2[i,c]
    nc.vector.memset(w2bd, 0.0)
    wd1bd = singles.tile([P, P], FP32)
    nc.vector.memset(wd1bd, 0.0)
    wd2bd = singles.tile([P, 12], FP32)      # lhsT[32g+i, 3g+o] = Wd2[o,i]
    nc.vector.memset(wd2bd, 0.0)
    for jj in range(8):
        for g in range(4):
            mprime = 4 * jj + g
            nc.sync.dma_start(
                out=w1q[jj][3 * mprime:3 * mprime + 3, 32 * g:32 * g + 32],
                in_=W1_d.rearrange("i c -> c i"))
    for raw_d, shp, bd, blk in ((W2_d, (32, 32), w2bd, 32),
                                (Wd1_d, (32, 32), wd1bd, 32),
                                (Wd2_d, (3, 32), wd2bd, 3)):
        raw = singles.tile(list(shp), FP32, tag=f"raw{id(bd)}")
        nc.sync.dma_start(out=raw, in_=raw_d)
        tps = psY.tile([shp[1], shp[0]], FP32, tag="ytp")
        nc.tensor.transpose(tps, raw, ident[:shp[0], :shp[0]])
        for g in range(4):
            nc.vector.tensor_copy(
                out=bd[32 * g:32 * g + 32, blk * g:blk * g + blk], in_=tps)

    w1qb = []
    for jj in range(8):
        t = singles.tile([96, P], BF16, tag=f"w1qb{jj}")
        nc.vector.tensor_copy(out=t, in_=w1q[jj])
        w1qb.append(t)
    w2bdb = singles.tile([P, P], BF16, tag="w2bdb")
    nc.vector.tensor_copy(out=w2bdb, in_=w2bd)
    wd1bdb = singles.tile([P, P], BF16, tag="wd1bdb")
    nc.vector.tensor_copy(out=wd1bdb, in_=wd1bd)
    wd2bdb = singles.tile([P, 12], BF16, tag="wd2bdb")
    nc.vector.tensor_copy(out=wd2bdb, in_=wd2bd)

    def bcast_vec(src_ap, n, reps, tag):   # DRAM [n] -> SBUF [reps*n, 1]
        t = singles.tile([reps * n, 1], FP32, tag=tag)
        src = bass.AP(tensor=src_ap.tensor, offset=src_ap.offset,
                      ap=[[0, reps], [1, n], [1, 1]])
        nc.sync.dma_start(out=t, in_=src)
        return t

    b1bd = bcast_vec(b1_d, 32, 4, "b1bd")     # [128,1]
    b2bd = bcast_vec(b2_d, 32, 4, "b2bd")
    bd1bd = bcast_vec(bd1_d, 32, 4, "bd1bd")
    # bd2 components broadcast to all partitions: [P,1] each
    bd2c = []
    for c in range(3):
        t = singles.tile([P, 1], FP32, tag=f"bd2c{c}")
        src = bass.AP(tensor=bd2_d.tensor, offset=bd2_d.offset + c,
                      ap=[[0, P], [1, 1]])
        nc.sync.dma_start(out=t, in_=src)
        bd2c.append(t)
    # world bias components broadcast to all partitions: bias_d is [1,3]
    bzc = []
    for c in range(3):
        t = singles.tile([P, 1], FP32, tag=f"bz{c}")
        src = bass.AP(tensor=bias_d.tensor, offset=bias_d.offset + c,
                      ap=[[0, P], [1, 1]])
        nc.sync.dma_start(out=t, in_=src)
        bzc.append(t)

    # ---------------- main loop ----------------
    v_bt = v_d.rearrange("(b p m) c -> b p (m c)", b=nbig, p=P)
    w_bt = w_d.rearrange("(b p m) c -> b p (m c)", b=nbig, p=P)
    o_bt = out_d.rearrange("(b p m) c -> b p (m c)", b=nbig, p=P)

    nmac = MG // 32

    from types import SimpleNamespace

    def front(bi):
        """DMA + geometry front-end for big-tile bi; returns live tiles."""
        v3i = geo.tile([P, MG, 3], FP32, tag="v3i")
        w3i = geo.tile([P, MG, 3], FP32, tag="w3i")
        nc.sync.dma_start(
            out=v3i.rearrange("p m c -> p (m c)"),
            in_=v_bt[bass.ds(bi, 1), :, :])
        nc.sync.dma_start(
            out=w3i.rearrange("p m c -> p (m c)"),
            in_=w_bt[bass.ds(bi, 1), :, :])

        if phase == "dmaonly":
            nc.sync.dma_start(
                out=o_bt[bass.ds(bi, 1), :, :],
                in_=v3i.rearrange("p m c -> p (m c)"))
            return None

        # --- deinterleave to planar bf16 (gpsimd), v/w planes duplicated so
        # the cross product can use contiguous shifted 3-plane views ---
        # X planes: [v0 v1 v2 v0 v1 v2 | w0 w1 w2 w0 w1 w2]
        X = geo.tile([P, 12, MG], BF16, tag="X")
        nc.gpsimd.tensor_copy(out=X[:, 0:3, :],
                              in_=v3i.rearrange("p m c -> p c m"))
        nc.gpsimd.tensor_copy(out=X[:, 6:9, :],
                              in_=w3i.rearrange("p m c -> p c m"))
        nc.vector.tensor_copy(out=X[:, 3:6, :], in_=X[:, 0:3, :])
        nc.vector.tensor_copy(out=X[:, 9:12, :], in_=X[:, 6:9, :])

        def xpl(k, n=3):   # planes {k..k+n}
            return bass.AP(tensor=X.tensor, offset=X.offset + k * MG,
                           ap=X.ap[:1] + [[MG, n], [1, MG]])

        def x2(k0, k1):    # planes {k0..k0+2, k1..k1+2} (regular 2x3 pattern)
            return bass.AP(tensor=X.tensor, offset=X.offset + k0 * MG,
                           ap=X.ap[:1] + [[(k1 - k0) * MG, 2], [MG, 3], [1, MG]])

        # --- squares / products / c-reductions ---
        Q = geo.tile([P, 9, MG], BF16, tag="Q")
        nc.scalar.activation(out=Q[:, 0:6, :], in_=x2(0, 6), func=AF.Square)
        nc.vector.tensor_mul(Q[:, 6:9, :], xpl(0), xpl(6))
        # triple-sum planes: R = [nvsq | nwsq | dt]; Q planes ordered
        # [v0 v1 v2 w0 w1 w2 p0 p1 p2]; sum t over planes {t, t+3, t+6}
        # is wrong -- we want {3t+k}: in_ = planes {k, k+3, k+6} gives
        # out t = plane (k + 3t), i.e. out = [vk wk pk]: so use stride 3MG.
        Rg = geo.tile([P, 3, MG], BF16, tag="Rg")

        def qk(k):   # planes {k, k+3, k+6}
            return bass.AP(tensor=Q.tensor, offset=Q.offset + k * MG,
                           ap=Q.ap[:1] + [[3 * MG, 3], [1, MG]])
        nc.vector.tensor_add(Rg, qk(0), qk(1))
        nc.vector.tensor_add(Rg, Rg, qk(2))
        nvsq = Rg[:, 0, :]
        nwsq = Rg[:, 1, :]
        dtp = Rg[:, 2, :]

        FEAT = geo.tile([P, MG, 3], BF16, tag="FEAT")
        nvf = geo.tile([P, MG], FP32, tag="nvf")
        nwf = geo.tile([P, MG], FP32, tag="nwf")
        s1 = geo.tile([P, MG], FP32, tag="s1")
        s2 = geo.tile([P, MG], FP32, tag="s2")
        nw2 = geo.tile([P, MG], BF16, tag="nw2")

        nc.scalar.activation(out=nvf, in_=nvsq, func=AF.Sqrt)
        nc.vector.reciprocal_approx_fast(out=s1, in_=nvf)
        # f1 -> FEAT[:,:,1] (bf16, strided)
        nc.vector.tensor_mul(FEAT[:, :, 1], dtp, s1)
        nc.vector._custom_dve(SUBSQ_CLAMP, out=nw2, in0=FEAT[:, :, 1],
                              in1=nwsq, s1=1e-5)
        nc.scalar.activation(out=nwf, in_=nw2, func=AF.Sqrt)
        nc.vector.reciprocal_approx_fast(out=s2, in_=nwf)
        nc.gpsimd.tensor_copy(out=FEAT[:, :, 0], in_=nvf)
        nc.gpsimd.tensor_copy(out=FEAT[:, :, 2], in_=nwf)
        s1b = geo.tile([P, MG], BF16, tag="s1b")
        s2b = geo.tile([P, MG], BF16, tag="s2b")
        ssb = geo.tile([P, MG], BF16, tag="ssb")
        nc.vector.tensor_copy(out=s1b, in_=s1)
        nc.vector.tensor_copy(out=s2b, in_=s2)
        nc.vector.tensor_mul(ssb, s1b, s2b)

        Y = geo.tile([P, MG, 3], BF16, tag="Y")
        return SimpleNamespace(bi=bi, X=X, xpl=xpl, FEAT=FEAT, Y=Y,
                               s1b=s1b, s2b=s2b, ssb=ssb)

    def back(G):
        """MLP macros (stage-major, IL-wide) + backend for big-tile G.bi."""
        bi, X, xpl, FEAT, Y = G.bi, G.X, G.xpl, G.FEAT, G.Y
        s1b, s2b, ssb = G.s1b, G.s2b, G.ssb

        # ---------------- MLP over 32-row macro-tiles ----------------
        if phase == "nomlp":
            nc.gpsimd.memset(Y, 0.0)

        def st_tl1(mk):
            m0 = 32 * mk
            gv = FEAT[:, m0:m0 + 32, :].rearrange("p m c -> p (m c)")
            ftp = psT.tile([96, P], BF16, tag="ftp")
            nc.tensor.transpose(ftp, gv, identb)
            rhs1 = mlp.tile([96, P], BF16, tag="rhs1")
            nc.vector.tensor_copy(out=rhs1, in_=ftp)
            h1ps = psM.tile([P, NM2], MMDT, tag="mm")
            for jj in range(8):
                nc.tensor.matmul(h1ps[:, P * jj:P * (jj + 1)],
                                 w1qb[jj], rhs1, start=True, stop=True)
            return h1ps

        def st_h1(mk, h1ps):
            h1 = mlp.tile([P, NM2], BF16, tag="h1")
            if H1_ACT > 0:
                nc.scalar.activation(out=h1[:, :H1_ACT], in_=h1ps[:, :H1_ACT],
                                     func=AF.Prelu, bias=b1bd, alpha=SLOPE)
            if H1_ACT < NM2:
                nc.vector._custom_dve(LEAKY_B, out=h1[:, H1_ACT:],
                                      in0=h1ps[:, H1_ACT:], s0=b1bd, s1=SLOPE)
            return h1

        def st_l2(mk, h1):
            ups = psM.tile([P, NM2], MMDT, tag="mm")
            for t in range(2):
                nc.tensor.matmul(ups[:, 512 * t:512 * (t + 1)], w2bdb,
                                 h1[:, 512 * t:512 * (t + 1)],
                                 start=True, stop=True)
            return ups

        def st_gate(mk, ups, h1):
            h = mlp.tile([P, NM2], BF16, tag="h")
            if G_ACT > 0:
                ul = mlp.tile([P, G_ACT], BF16, tag="ul")
                nc.scalar.activation(out=ul, in_=ups[:, :G_ACT],
                                     func=AF.Prelu, bias=b2bd, alpha=SLOPE)
                nc.vector.tensor_mul(h[:, :G_ACT], ul, h1[:, :G_ACT])
            if G_ACT < NM2:
                nc.vector._custom_dve(GATE_LEAKY_B, out=h[:, G_ACT:],
                                      in0=ups[:, G_ACT:], in1=h1[:, G_ACT:],
                                      s0=b2bd, s1=SLOPE)
            return h

        def st_l3(mk, h):
            y1ps = psM.tile([P, NM2], MMDT, tag="mm")
            for t in range(2):
                nc.tensor.matmul(y1ps[:, 512 * t:512 * (t + 1)], wd1bdb,
                                 h[:, 512 * t:512 * (t + 1)],
                                 start=True, stop=True)
            return y1ps

        def st_y1(mk, y1ps):
            y1 = mlp.tile([P, NM2], BF16, tag="y1")
            if Y1_ACT > 0:
                nc.scalar.activation(out=y1[:, :Y1_ACT], in_=y1ps[:, :Y1_ACT],
                                     func=AF.Prelu, bias=bd1bd, alpha=SLOPE)
            if Y1_ACT < NM2:
                nc.vector._custom_dve(LEAKY_B, out=y1[:, Y1_ACT:],
                                      in0=y1ps[:, Y1_ACT:], s0=bd1bd, s1=SLOPE)
            return y1

        def st_out(mk, y1):
            m0 = 32 * mk
            ytp = psY.tile([P, 96], FP32, tag="ytp")
            for jj in range(8):
                nc.tensor.matmul(ytp[:, 12 * jj:12 * (jj + 1)],
                                 y1[:, P * jj:P * (jj + 1)], wd2bdb,
                                 start=True, stop=True)
            # plain evacuation; bd2 is folded into the backend STT ops
            yv = Y[:, m0:m0 + 32, :].rearrange("p m c -> p (m c)")
            nc.scalar.activation(out=yv, in_=ytp, func=AF.Copy)

        if phase != "nomlp":
            for mk0 in range(0, nmac, IL):
                mks = [mk0 + d for d in range(IL) if mk0 + d < nmac]
                st = {}
                for mk in mks:
                    st[mk] = [st_tl1(mk)]
                for mk in mks:
                    st[mk].append(st_h1(mk, st[mk][-1]))
                for mk in mks:
                    st[mk].append(st_l2(mk, st[mk][-1]))
                for mk in mks:
                    st[mk].append(st_gate(mk, st[mk][-1], st[mk][-2]))
                for mk in mks:
                    st[mk].append(st_l3(mk, st[mk][-1]))
                for mk in mks:
                    st[mk].append(st_y1(mk, st[mk][-1]))
                for mk in mks:
                    st_out(mk, st[mk][-1])

        # ---------------- back-end rotation ----------------
        bp = geo.tile([P, MG], BF16, tag="bp")
        ap_ = geo.tile([P, MG], BF16, tag="ap")
        cp = geo.tile([P, MG], BF16, tag="cp")
        tt = geo.tile([P, MG], BF16, tag="tt")
        uu = geo.tile([P, MG], BF16, tag="uu")
        # b = (y1 + bd2_1) * s2
        nc.vector.scalar_tensor_tensor(out=bp, in0=Y[:, :, 1], scalar=bd2c[1],
                                       in1=s2b, op0=ALU.add, op1=ALU.mult)
        # t = b * f1
        nc.vector.tensor_mul(tt, bp, FEAT[:, :, 1])
        # u = (y0 + bd2_0) - t
        nc.vector.scalar_tensor_tensor(out=uu, in0=Y[:, :, 0], scalar=bd2c[0],
                                       in1=tt, op0=ALU.add, op1=ALU.subtract)
        # a = u * s1
        nc.vector.tensor_mul(ap_, uu, s1b)
        # c = (y2 + bd2_2) * s1 * s2
        nc.vector.scalar_tensor_tensor(out=cp, in0=Y[:, :, 2], scalar=bd2c[2],
                                       in1=ssb, op0=ALU.add, op1=ALU.mult)

        # cross product via shifted duplicated-plane views:
        # cv_c = v_{c+1} w_{c+2} - v_{c+2} w_{c+1}  (indices mod 3)
        # X[1:4]=[v1 v2 v0], X[8:11]=[w2 w0 w1], X[2:5]=[v2 v0 v1], X[7:10]=[w1 w2 w0]
        cv = geo.tile([P, 3, MG], BF16, tag="cv")
        qq = geo.tile([P, 3, MG], BF16, tag="qq")
        nc.gpsimd.tensor_mul(qq, xpl(1), xpl(8))
        nc.vector.tensor_mul(cv, xpl(2), xpl(7))
        nc.vector.tensor_sub(cv, qq, cv)

        # A3 = a*v, B3 = b*w, S = A3+B3, R3 = c*cv  (3-plane-wide, bcast scalar)
        def bcast(t):
            return bass.AP(tensor=t.tensor, offset=t.offset,
                           ap=t.ap[:1] + [[0, 3], [1, MG]])
        A3 = geo.tile([P, 3, MG], BF16, tag="A3")
        B3 = geo.tile([P, 3, MG], BF16, tag="B3")
        nc.vector.tensor_mul(A3, xpl(0), bcast(ap_))
        nc.gpsimd.tensor_mul(B3, xpl(6), bcast(bp))
        nc.vector.tensor_add(A3, A3, B3)
        nc.vector.tensor_mul(cv, cv, bcast(cp))

        # out_c = (A3_c + bias_c) + cv_c, interleaved fp32
        oby = geo.tile([P, MG, 3], FP32, tag="oby")
        for c in range(3):
            nc.vector.scalar_tensor_tensor(
                out=oby[:, :, c], in0=A3[:, c, :], scalar=bzc[c],
                in1=cv[:, c, :], op0=ALU.add, op1=ALU.add)

        nc.sync.dma_start(
            out=o_bt[bass.ds(bi, 1), :, :],
            in_=oby.rearrange("p m c -> p (m c)"))

    def one_pass():
        pend = None
        for b in range(nbig):
            g = front(b)
            if phase == "dmaonly":
                continue
            if pend is not None:
                back(pend)
            pend = g
        if pend is not None:
            back(pend)

    if LOOP_MODE == "python":          # fully unrolled (for TimelineSim)
        for _ in range(repeat):
            one_pass()
    else:
        with tc.For_i(0, repeat, 1):
            one_pass()


def _build(nbig, repeat=1, phase='full'):
    nc = bacc.Bacc("TRN2", target_bir_lowering=False, debug=False,
                   num_devices=NCORES)
    rows = nbig * ROWS_BIG
    v_d = nc.dram_tensor("v", [rows, 3], FP32, kind="ExternalInput").ap()
    w_d = nc.dram_tensor("w", [rows, 3], FP32, kind="ExternalInput").ap()
    W1_d = nc.dram_tensor("W1", [32, 3], FP32, kind="ExternalInput").ap()
    b1_d = nc.dram_tensor("b1", [32], FP32, kind="ExternalInput").ap()
    W2_d = nc.dram_tensor("W2", [32, 32], FP32, kind="ExternalInput").ap()
    b2_d = nc.dram_tensor("b2", [32], FP32, kind="ExternalInput").ap()
    Wd1_d = nc.dram_tensor("Wd1", [32, 32], FP32, kind="ExternalInput").ap()
    bd1_d = nc.dram_tensor("bd1", [32], FP32, kind="ExternalInput").ap()
    Wd2_d = nc.dram_tensor("Wd2", [3, 32], FP32, kind="ExternalInput").ap()
    bd2_d = nc.dram_tensor("bd2", [3], FP32, kind="ExternalInput").ap()
    bias_d = nc.dram_tensor("bias", [1, 3], FP32, kind="ExternalInput").ap()
    out_d = nc.dram_tensor("out", [rows, 3], FP32, kind="ExternalOutput").ap()

    wts = (W1_d, b1_d, W2_d, b2_d, Wd1_d, bd1_d, Wd2_d, bd2_d, bias_d)
    with tile.TileContext(nc) as tc:
        with ExitStack() as ctx:
            _body(ctx, tc, nbig, v_d, w_d, wts, out_d, repeat, phase)
    nc.compile()
    return nc


_NC_CACHE = {}


def _get_nc(nbig, repeat=1, phase="full"):
    key = (nbig, repeat, phase, LOOP_MODE, PSM_BUFS, PST_BUFS, PSY_BUFS,
           GEO_BUFS, MLP_BUFS, H1_ACT, Y1_ACT, G_ACT, OBY_POOL, IL, PSUM_BF16)
    if key not in _NC_CACHE:
        _NC_CACHE[key] = _build(nbig, repeat, phase)
    return _NC_CACHE[key]


WNAMES = ["W1", "b1", "W2", "b2", "Wd1", "bd1", "Wd2", "bd2", "bias"]


def _run(v, w, wdict, nbig, n_cores, trace=False, repeat=1, phase="full"):
    nc = _get_nc(nbig, repeat, phase)
    rows = nbig * ROWS_BIG
    in_maps = []
    for c in range(n_cores):
        m = {"v": np.ascontiguousarray(v[c * rows:(c + 1) * rows]),
             "w": np.ascontiguousarray(w[c * rows:(c + 1) * rows])}
        for k in WNAMES:
            m[k] = wdict[k]
        in_maps.append(m)
    last_err = None
    for attempt in range(3):
        try:
            res = run_bass_kernel_spmd(nc, in_maps,
                                       core_ids=list(range(n_cores)),
                                       trace=trace)
            break
        except Exception as e:      # transient NRT device errors
            last_err = e
            import time as _t
            _t.sleep(5)
    else:
        raise last_err
    out = np.concatenate([res.results[c]["out"] for c in range(n_cores)], axis=0)
    return out, res


def kernel(**inputs):
    v = np.ascontiguousarray(np.asarray(inputs["v"], dtype=np.float32))
    w = np.ascontiguousarray(np.asarray(inputs["w"], dtype=np.float32))
    wdict = {k: np.ascontiguousarray(np.asarray(inputs[k], dtype=np.float32))
             for k in WNAMES}
    wdict["bias"] = wdict["bias"].reshape(1, 3)
    out, _ = _run(v, w, wdict, BC // ROWS_BIG, NCORES)
    return out


# revision 15
# speedup vs baseline: 2.9984x; 2.9984x over previous
"""AeroModel (gram-schmidt frame + tiny MLP) Trainium2 kernel, v2.

Self-contained: hardcodes shapes B=2097152, H=32, 8-core data-parallel sharding.
kernel(**inputs) takes full unsharded inputs, returns full [B,3] float32 output.

Math (equivalent to the reference, avoids materializing the rotation matrix):
    nv  = |v|            s1 = 1/nv
    dt  = v.w            f1 = dt*s1            (= w . v_on)
    nw2 = |w|^2 - f1^2   nw = sqrt(nw2)        (= |w_orth|)   s2 = 1/nw
    feat = [nv, f1, nw]
    y = MLP(feat)        (H=32, leaky-relu 0.01, gated 2nd layer)
    out = a*v + b*w + c*(v x w) + bias
      where b = y1*s2, a = s1*(y0 - b*f1), c = y2*s1*s2

v2 structure vs v1:
  - custom DVE ops: LEAKY_B (fused bias+leaky, one op) and GATE_LEAKY_B
    (fused bias+leaky+gating mul, one op) collapse the MLP evacuations.
  - planar bf16 geometry: v/w deinterleaved once on gpsimd, all elementwise
    at DVE 2x bf16 rate; 3 reductions done with two 3-plane-wide adds.
  - MG=512 big-tiles with double-buffered geo pool so big-tile N+1's
    DMA/front-end overlaps N's MLP/backend.
"""
import os
import numpy as np
from contextlib import ExitStack

import concourse.bass as bass
import concourse.tile as tile
from concourse import bacc, mybir
from concourse.bass_utils import run_bass_kernel_spmd
from concourse.masks import make_identity

from concourse.dve_ops import DveOp, OPS, CUSTOM_DVE_SPECS, _SUB_OPCODE_FOR_NAME
from concourse.dve_spec import Spec, Src0, Src1, C0, C1, maxx, sq, lower
from concourse.dve_uop import DveOpSpec

AF = mybir.ActivationFunctionType
ALU = mybir.AluOpType
FP32 = mybir.dt.float32
BF16 = mybir.dt.bfloat16

B = 2097152
NCORES = 8
BC = B // NCORES          # rows per core
P = 128
MG = int(os.environ.get("K_MG", "512"))   # rows per partition per big-tile
ROWS_BIG = P * MG
NM2 = 1024                # MLP macro columns (32 m-values x 32 hidden)
SLOPE = 0.01
LOOP_MODE = os.environ.get("K_LOOP", "plain")
GEO_BUFS = int(os.environ.get("K_GEOBUFS", "2"))
MLP_BUFS = int(os.environ.get("K_MLPBUFS", "3"))
PSM_BUFS = int(os.environ.get("K_PSM", "3"))
PST_BUFS = int(os.environ.get("K_PST", "1"))
PSY_BUFS = int(os.environ.get("K_PSY", "1"))
# engine split knobs: ACT share (columns out of 1024) for each MLP layer
H1_ACT = int(os.environ.get("K_H1ACT", "1024"))  # h1: rest on DVE custom
G_ACT = int(os.environ.get("K_GACT", "0"))       # gate: ACT prelu + DVE mul share
OBY_POOL = int(os.environ.get("K_OBYPOOL", "2")) # oby STT planes on Pool
IL = int(os.environ.get("K_IL", "2"))            # macro interleave width
Y1_ACT = int(os.environ.get("K_Y1ACT", "512"))   # y1: rest on DVE custom
PSUM_BF16 = int(os.environ.get("K_PSUMBF16", "0"))


# ---------------- custom DVE ops ----------------
def _register_op(name, spec, subdim=False):
    if name in _SUB_OPCODE_FOR_NAME:
        for op in OPS:
            if op.name == name:
                return op
    shas = {}
    for ver in ("v3", "v4"):
        uops = lower(spec, ver=ver)
        shas[ver] = DveOpSpec(name=name, opcode=1, uops=uops).sha(ver)
    op = DveOp(name, spec, subdim=subdim, uops_sha=shas)
    OPS.append(op)
    CUSTOM_DVE_SPECS[op.name] = op.spec
    _SUB_OPCODE_FOR_NAME[op.name] = max(_SUB_OPCODE_FOR_NAME.values()) + 1
    assert _SUB_OPCODE_FOR_NAME[op.name] < 0x20
    return op


_t = Src0 + C0
LEAKY_B = _register_op(
    "LEAKY_B",
    Spec(body=maxx(_t, _t * C1),
         reference=lambda in0, in1, s0, s1, imm2: np.maximum(
             in0.astype(np.float32) + s0, (in0.astype(np.float32) + s0) * s1)),
)
GATE_LEAKY_B = _register_op(
    "GATE_LEAKY_B",
    Spec(body=maxx(_t, _t * C1) * Src1,
         reference=lambda in0, in1, s0, s1, imm2: np.maximum(
             in0.astype(np.float32) + s0,
             (in0.astype(np.float32) + s0) * s1) * in1),
)
# nw2 = max(nwsq - f1^2, eps): guards Sqrt against bf16-rounding negatives
SUBSQ_CLAMP = _register_op(
    "SUBSQ_CLAMP",
    Spec(body=maxx(Src1 - sq(Src0), C1),
         reference=lambda in0, in1, s0, s1, imm2: np.maximum(
             in1.astype(np.float32) - in0.astype(np.float32) ** 2, s1)),
)


def _body(ctx, tc, nbig, v_d, w_d, wts, out_d, repeat=1, phase='full'):
    nc = tc.nc
    W1_d, b1_d, W2_d, b2_d, Wd1_d, bd1_d, Wd2_d, bd2_d, bias_d = wts
    MMDT = BF16 if PSUM_BF16 else FP32

    singles = ctx.enter_context(tc.tile_pool(name="singles", bufs=1))
    geo = ctx.enter_context(tc.tile_pool(name="geo", bufs=GEO_BUFS))
    mlp = ctx.enter_context(tc.tile_pool(name="mlp", bufs=MLP_BUFS))
    psT = ctx.enter_context(tc.tile_pool(name="psT", bufs=PST_BUFS, space="PSUM"))
    psM = ctx.enter_context(tc.tile_pool(name="psM", bufs=PSM_BUFS, space="PSUM"))
    psY = ctx.enter_context(tc.tile_pool(name="psY", bufs=PSY_BUFS, space="PSUM"))

    # ---------------- one-time prep ----------------
    ident = singles.tile([P, P], FP32)
    make_identity(nc, ident)
    identb = singles.tile([P, P], BF16, tag="identb")
    nc.vector.tensor_copy(out=identb, in_=ident)

    # block-diagonal lhsT weights (4 groups); transposes done on PE
    w1q = []
    for jj in range(8):
        t = singles.tile([96, P], FP32, tag=f"w1q{jj}")
        nc.vector.memset(t, 0.0)
        w1q.append(t)
    w2bd = singles.tile([P, P], FP32)        # lhsT[32g+c, 32g+i] = W2[i,c]
    nc.vector.memset(w2bd, 0.0)
    wd1bd = singles.tile([P, P], FP32)
    nc.vector.memset(wd1bd, 0.0)
    wd2bd = singles.tile([P, 12], FP32)      # lhsT[32g+i, 3g+o] = Wd2[o,i]
    nc.vector.memset(wd2bd, 0.0)
    for jj in range(8):
        for g in range(4):
            mprime = 4 * jj + g
            nc.sync.dma_start(
                out=w1q[jj][3 * mprime:3 * mprime + 3, 32 * g:32 * g + 32],
                in_=W1_d.rearrange("i c -> c i"))
    for raw_d, shp, bd, blk in ((W2_d, (32, 32), w2bd, 32),
                                (Wd1_d, (32, 32), wd1bd, 32),
                                (Wd2_d, (3, 32), wd2bd, 3)):
        raw = singles.tile(list(shp), FP32, tag=f"raw{id(bd)}")
        nc.sync.dma_start(out=raw, in_=raw_d)
        tps = psY.tile([shp[1], shp[0]], FP32, tag="ytp")
        nc.tensor.transpose(tps, raw, ident[:shp[0], :shp[0]])
        for g in range(4):
            nc.vector.tensor_copy(
                out=bd[32 * g:32 * g + 32, blk * g:blk * g + blk], in_=tps)

    w1qb = []
    for jj in range(8):
        t = singles.tile([96, P], BF16, tag=f"w1qb{jj}")
        nc.vector.tensor_copy(out=t, in_=w1q[jj])
        w1qb.append(t)
    w2bdb = singles.tile([P, P], BF16, tag="w2bdb")
    nc.vector.tensor_copy(out=w2bdb, in_=w2bd)
    wd1bdb = singles.tile([P, P], BF16, tag="wd1bdb")
    nc.vector.tensor_copy(out=wd1bdb, in_=wd1bd)
    wd2bdb = singles.tile([P, 12], BF16, tag="wd2bdb")
    nc.vector.tensor_copy(out=wd2bdb, in_=wd2bd)

    def bcast_vec(src_ap, n, reps, tag):   # DRAM [n] -> SBUF [reps*n, 1]
        t = singles.tile([reps * n, 1], FP32, tag=tag)
        src = bass.AP(tensor=src_ap.tensor, offset=src_ap.offset,
                      ap=[[0, reps], [1, n], [1, 1]])
        nc.sync.dma_start(out=t, in_=src)
        return t

    b1bd = bcast_vec(b1_d, 32, 4, "b1bd")     # [128,1]
    b2bd = bcast_vec(b2_d, 32, 4, "b2bd")
    bd1bd = bcast_vec(bd1_d, 32, 4, "bd1bd")
    # bd2 components broadcast to all partitions: [P,1] each
    bd2c = []
    for c in range(3):
        t = singles.tile([P, 1], FP32, tag=f"bd2c{c}")
        src = bass.AP(tensor=bd2_d.tensor, offset=bd2_d.offset + c,
                      ap=[[0, P], [1, 1]])
        nc.sync.dma_start(out=t, in_=src)
        bd2c.append(t)
    # world bias components broadcast to all partitions: bias_d is [1,3]
    bzc = []
    for c in range(3):
        t = singles.tile([P, 1], FP32, tag=f"bz{c}")
        src = bass.AP(tensor=bias_d.tensor, offset=bias_d.offset + c,
                      ap=[[0, P], [1, 1]])
        nc.sync.dma_start(out=t, in_=src)
        bzc.append(t)

    # ---------------- main loop ----------------
    v_bt = v_d.rearrange("(b p m) c -> b p (m c)", b=nbig, p=P)
    w_bt = w_d.rearrange("(b p m) c -> b p (m c)", b=nbig, p=P)
    o_bt = out_d.rearrange("(b p m) c -> b p (m c)", b=nbig, p=P)

    nmac = MG // 32

    from types import SimpleNamespace

    def front(bi):
        """DMA + geometry front-end for big-tile bi; returns live tiles."""
        v3i = geo.tile([P, MG, 3], FP32, tag="v3i")
        w3i = geo.tile([P, MG, 3], FP32, tag="w3i")
        nc.sync.dma_start(
            out=v3i.rearrange("p m c -> p (m c)"),
            in_=v_bt[bass.ds(bi, 1), :, :])
        nc.sync.dma_start(
            out=w3i.rearrange("p m c -> p (m c)"),
            in_=w_bt[bass.ds(bi, 1), :, :])

        if phase == "dmaonly":
            nc.sync.dma_start(
                out=o_bt[bass.ds(bi, 1), :, :],
                in_=v3i.rearrange("p m c -> p (m c)"))
            return None

        # --- deinterleave to planar bf16 (gpsimd), v/w planes duplicated so
        # the cross product can use contiguous shifted 3-plane views ---
        # X planes: [v0 v1 v2 v0 v1 v2 | w0 w1 w2 w0 w1 w2]
        X = geo.tile([P, 12, MG], BF16, tag="X")
        nc.gpsimd.tensor_copy(out=X[:, 0:3, :],
                              in_=v3i.rearrange("p m c -> p c m"))
        nc.gpsimd.tensor_copy(out=X[:, 6:9, :],
                              in_=w3i.rearrange("p m c -> p c m"))
        return SimpleNamespace(bi=bi, X=X)

    def front_chain(G):
        """DVE/ACT geometry chain; emit AFTER the previous tile's MLP so its
        serial ACT round-trips don't block the DVE queue head."""
        bi, X = G.bi, G.X
        nc.vector.tensor_copy(out=X[:, 3:6, :], in_=X[:, 0:3, :])
        nc.vector.tensor_copy(out=X[:, 9:12, :], in_=X[:, 6:9, :])

        def xpl(k, n=3):   # planes {k..k+n}
            return bass.AP(tensor=X.tensor, offset=X.offset + k * MG,
                           ap=X.ap[:1] + [[MG, n], [1, MG]])

        def x2(k0, k1):    # planes {k0..k0+2, k1..k1+2} (regular 2x3 pattern)
            return bass.AP(tensor=X.tensor, offset=X.offset + k0 * MG,
                           ap=X.ap[:1] + [[(k1 - k0) * MG, 2], [MG, 3], [1, MG]])

        # --- squares / products / c-reductions ---
        Q = geo.tile([P, 9, MG], BF16, tag="Q")
        nc.scalar.activation(out=Q[:, 0:6, :], in_=x2(0, 6), func=AF.Square)
        nc.vector.tensor_mul(Q[:, 6:9, :], xpl(0), xpl(6))
        # triple-sum planes: R = [nvsq | nwsq | dt]; Q planes ordered
        # [v0 v1 v2 w0 w1 w2 p0 p1 p2]; sum t over planes {t, t+3, t+6}
        # is wrong -- we want {3t+k}: in_ = planes {k, k+3, k+6} gives
        # out t = plane (k + 3t), i.e. out = [vk wk pk]: so use stride 3MG.
        Rg = geo.tile([P, 3, MG], BF16, tag="Rg")

        def qk(k):   # planes {k, k+3, k+6}
            return bass.AP(tensor=Q.tensor, offset=Q.offset + k * MG,
                           ap=Q.ap[:1] + [[3 * MG, 3], [1, MG]])
        nc.vector.tensor_add(Rg, qk(0), qk(1))
        nc.vector.tensor_add(Rg, Rg, qk(2))
        nvsq = Rg[:, 0, :]
        nwsq = Rg[:, 1, :]
        dtp = Rg[:, 2, :]

        FEAT = geo.tile([P, MG, 3], BF16, tag="FEAT")
        nvf = geo.tile([P, MG], FP32, tag="nvf")
        nwf = geo.tile([P, MG], FP32, tag="nwf")
        s1 = geo.tile([P, MG], FP32, tag="s1")
        s2 = geo.tile([P, MG], FP32, tag="s2")
        nw2 = geo.tile([P, MG], BF16, tag="nw2")

        nc.scalar.activation(out=nvf, in_=nvsq, func=AF.Sqrt)
        nc.vector.reciprocal_approx_fast(out=s1, in_=nvf)
        # f1 -> FEAT[:,:,1] (bf16, strided)
        nc.vector.tensor_mul(FEAT[:, :, 1], dtp, s1)
        nc.vector._custom_dve(SUBSQ_CLAMP, out=nw2, in0=FEAT[:, :, 1],
                              in1=nwsq, s1=1e-5)
        nc.scalar.activation(out=nwf, in_=nw2, func=AF.Sqrt)
        nc.vector.reciprocal_approx_fast(out=s2, in_=nwf)
        nc.gpsimd.tensor_copy(out=FEAT[:, :, 0], in_=nvf)
        nc.gpsimd.tensor_copy(out=FEAT[:, :, 2], in_=nwf)
        s1b = geo.tile([P, MG], BF16, tag="s1b")
        s2b = geo.tile([P, MG], BF16, tag="s2b")
        ssb = geo.tile([P, MG], BF16, tag="ssb")
        nc.vector.tensor_copy(out=s1b, in_=s1)
        nc.vector.tensor_copy(out=s2b, in_=s2)
        nc.vector.tensor_mul(ssb, s1b, s2b)

        Y = geo.tile([P, MG, 3], BF16, tag="Y")
        G.xpl, G.FEAT, G.Y = xpl, FEAT, Y
        G.s1b, G.s2b, G.ssb = s1b, s2b, ssb
        return G

    def back(G):
        """MLP macros (stage-major, IL-wide) + backend for big-tile G.bi."""
        bi, X, xpl, FEAT, Y = G.bi, G.X, G.xpl, G.FEAT, G.Y
        s1b, s2b, ssb = G.s1b, G.s2b, G.ssb

        # ---------------- MLP over 32-row macro-tiles ----------------
        if phase == "nomlp":
            nc.gpsimd.memset(Y, 0.0)

        def st_tl1(mk):
            m0 = 32 * mk
            gv = FEAT[:, m0:m0 + 32, :].rearrange("p m c -> p (m c)")
            ftp = psT.tile([96, P], BF16, tag="ftp")
            nc.tensor.transpose(ftp, gv, identb)
            rhs1 = mlp.tile([96, P], BF16, tag="rhs1")
            nc.vector.tensor_copy(out=rhs1, in_=ftp)
            h1ps = psM.tile([P, NM2], MMDT, tag="mm")
            for jj in range(8):
                nc.tensor.matmul(h1ps[:, P * jj:P * (jj + 1)],
                                 w1qb[jj], rhs1, start=True, stop=True)
            return h1ps

        def st_h1(mk, h1ps):
            h1 = mlp.tile([P, NM2], BF16, tag="h1")
            if H1_ACT > 0:
                nc.scalar.activation(out=h1[:, :H1_ACT], in_=h1ps[:, :H1_ACT],
                                     func=AF.Prelu, bias=b1bd, alpha=SLOPE)
            if H1_ACT < NM2:
                nc.vector._custom_dve(LEAKY_B, out=h1[:, H1_ACT:],
                                      in0=h1ps[:, H1_ACT:], s0=b1bd, s1=SLOPE)
            return h1

        def st_l2(mk, h1):
            ups = psM.tile([P, NM2], MMDT, tag="mm")
            for t in range(2):
                nc.tensor.matmul(ups[:, 512 * t:512 * (t + 1)], w2bdb,
                                 h1[:, 512 * t:512 * (t + 1)],
                                 start=True, stop=True)
            return ups

        def st_gate(mk, ups, h1):
            h = mlp.tile([P, NM2], BF16, tag="h")
            if G_ACT > 0:
                ul = mlp.tile([P, G_ACT], BF16, tag="ul")
                nc.scalar.activation(out=ul, in_=ups[:, :G_ACT],
                                     func=AF.Prelu, bias=b2bd, alpha=SLOPE)
                nc.vector.tensor_mul(h[:, :G_ACT], ul, h1[:, :G_ACT])
            if G_ACT < NM2:
                nc.vector._custom_dve(GATE_LEAKY_B, out=h[:, G_ACT:],
                                      in0=ups[:, G_ACT:], in1=h1[:, G_ACT:],
                                      s0=b2bd, s1=SLOPE)
            return h

        def st_l3(mk, h):
            y1ps = psM.tile([P, NM2], MMDT, tag="mm")
            for t in range(2):
                nc.tensor.matmul(y1ps[:, 512 * t:512 * (t + 1)], wd1bdb,
                                 h[:, 512 * t:512 * (t + 1)],
                                 start=True, stop=True)
            return y1ps

        def st_y1(mk, y1ps):
            y1 = mlp.tile([P, NM2], BF16, tag="y1")
            if Y1_ACT > 0:
                nc.scalar.activation(out=y1[:, :Y1_ACT], in_=y1ps[:, :Y1_ACT],
                                     func=AF.Prelu, bias=bd1bd, alpha=SLOPE)
            if Y1_ACT < NM2:
                nc.vector._custom_dve(LEAKY_B, out=y1[:, Y1_ACT:],
                                      in0=y1ps[:, Y1_ACT:], s0=bd1bd, s1=SLOPE)
            return y1

        def st_out(mk, y1):
            m0 = 32 * mk
            ytp = psY.tile([P, 96], FP32, tag="ytp")
            for jj in range(8):
                nc.tensor.matmul(ytp[:, 12 * jj:12 * (jj + 1)],
                                 y1[:, P * jj:P * (jj + 1)], wd2bdb,
                                 start=True, stop=True)
            # plain evacuation; bd2 is folded into the backend STT ops
            yv = Y[:, m0:m0 + 32, :].rearrange("p m c -> p (m c)")
            nc.scalar.activation(out=yv, in_=ytp, func=AF.Copy)

        if phase != "nomlp":
            for mk0 in range(0, nmac, IL):
                mks = [mk0 + d for d in range(IL) if mk0 + d < nmac]
                st = {}
                for mk in mks:
                    st[mk] = [st_tl1(mk)]
                for mk in mks:
                    st[mk].append(st_h1(mk, st[mk][-1]))
                for mk in mks:
                    st[mk].append(st_l2(mk, st[mk][-1]))
                for mk in mks:
                    st[mk].append(st_gate(mk, st[mk][-1], st[mk][-2]))
                for mk in mks:
                    st[mk].append(st_l3(mk, st[mk][-1]))
                for mk in mks:
                    st[mk].append(st_y1(mk, st[mk][-1]))
                for mk in mks:
                    st_out(mk, st[mk][-1])

        # ---------------- back-end rotation ----------------
        bp = geo.tile([P, MG], BF16, tag="bp")
        ap_ = geo.tile([P, MG], BF16, tag="ap")
        cp = geo.tile([P, MG], BF16, tag="cp")
        tt = geo.tile([P, MG], BF16, tag="tt")
        uu = geo.tile([P, MG], BF16, tag="uu")
        # b = (y1 + bd2_1) * s2
        nc.vector.scalar_tensor_tensor(out=bp, in0=Y[:, :, 1], scalar=bd2c[1],
                                       in1=s2b, op0=ALU.add, op1=ALU.mult)
        # t = b * f1
        nc.vector.tensor_mul(tt, bp, FEAT[:, :, 1])
        # u = (y0 + bd2_0) - t
        nc.vector.scalar_tensor_tensor(out=uu, in0=Y[:, :, 0], scalar=bd2c[0],
                                       in1=tt, op0=ALU.add, op1=ALU.subtract)
        # a = u * s1
        nc.vector.tensor_mul(ap_, uu, s1b)
        # c = (y2 + bd2_2) * s1 * s2
        nc.vector.scalar_tensor_tensor(out=cp, in0=Y[:, :, 2], scalar=bd2c[2],
                                       in1=ssb, op0=ALU.add, op1=ALU.mult)

        # cross product via shifted duplicated-plane views:
        # cv_c = v_{c+1} w_{c+2} - v_{c+2} w_{c+1}  (indices mod 3)
        # X[1:4]=[v1 v2 v0], X[8:11]=[w2 w0 w1], X[2:5]=[v2 v0 v1], X[7:10]=[w1 w2 w0]
        cv = geo.tile([P, 3, MG], BF16, tag="cv")
        qq = geo.tile([P, 3, MG], BF16, tag="qq")
        nc.gpsimd.tensor_mul(qq, xpl(1), xpl(8))
        nc.vector.tensor_mul(cv, xpl(2), xpl(7))
        nc.vector.tensor_sub(cv, qq, cv)

        # A3 = a*v, B3 = b*w, S = A3+B3, R3 = c*cv  (3-plane-wide, bcast scalar)
        def bcast(t):
            return bass.AP(tensor=t.tensor, offset=t.offset,
                           ap=t.ap[:1] + [[0, 3], [1, MG]])
        A3 = geo.tile([P, 3, MG], BF16, tag="A3")
        B3 = geo.tile([P, 3, MG], BF16, tag="B3")
        nc.vector.tensor_mul(A3, xpl(0), bcast(ap_))
        nc.gpsimd.tensor_mul(B3, xpl(6), bcast(bp))
        nc.vector.tensor_add(A3, A3, B3)
        nc.vector.tensor_mul(cv, cv, bcast(cp))

        # out_c = (A3_c + bias_c) + cv_c, interleaved fp32
        oby = geo.tile([P, MG, 3], FP32, tag="oby")
        for c in range(3):
            nc.vector.scalar_tensor_tensor(
                out=oby[:, :, c], in0=A3[:, c, :], scalar=bzc[c],
                in1=cv[:, c, :], op0=ALU.add, op1=ALU.add)

        nc.sync.dma_start(
            out=o_bt[bass.ds(bi, 1), :, :],
            in_=oby.rearrange("p m c -> p (m c)"))

    def one_pass():
        pend = None
        for b in range(nbig):
            g = front(b)
            if phase == "dmaonly":
                continue
            if pend is not None:
                back(pend)
            front_chain(g)
            pend = g
        if pend is not None:
            back(pend)

    if LOOP_MODE == "python":          # fully unrolled (for TimelineSim)
        for _ in range(repeat):
            one_pass()
    else:
        with tc.For_i(0, repeat, 1):
            one_pass()


def _build(nbig, repeat=1, phase='full'):
    nc = bacc.Bacc("TRN2", target_bir_lowering=False, debug=False,
                   num_devices=NCORES)
    rows = nbig * ROWS_BIG
    v_d = nc.dram_tensor("v", [rows, 3], FP32, kind="ExternalInput").ap()
    w_d = nc.dram_tensor("w", [rows, 3], FP32, kind="ExternalInput").ap()
    W1_d = nc.dram_tensor("W1", [32, 3], FP32, kind="ExternalInput").ap()
    b1_d = nc.dram_tensor("b1", [32], FP32, kind="ExternalInput").ap()
    W2_d = nc.dram_tensor("W2", [32, 32], FP32, kind="ExternalInput").ap()
    b2_d = nc.dram_tensor("b2", [32], FP32, kind="ExternalInput").ap()
    Wd1_d = nc.dram_tensor("Wd1", [32, 32], FP32, kind="ExternalInput").ap()
    bd1_d = nc.dram_tensor("bd1", [32], FP32, kind="ExternalInput").ap()
    Wd2_d = nc.dram_tensor("Wd2", [3, 32], FP32, kind="ExternalInput").ap()
    bd2_d = nc.dram_tensor("bd2", [3], FP32, kind="ExternalInput").ap()
    bias_d = nc.dram_tensor("bias", [1, 3], FP32, kind="ExternalInput").ap()
    out_d = nc.dram_tensor("out", [rows, 3], FP32, kind="ExternalOutput").ap()

    wts = (W1_d, b1_d, W2_d, b2_d, Wd1_d, bd1_d, Wd2_d, bd2_d, bias_d)
    with tile.TileContext(nc) as tc:
        with ExitStack() as ctx:
            _body(ctx, tc, nbig, v_d, w_d, wts, out_d, repeat, phase)
    nc.compile()
    return nc


_NC_CACHE = {}


def _get_nc(nbig, repeat=1, phase="full"):
    key = (nbig, repeat, phase, LOOP_MODE, PSM_BUFS, PST_BUFS, PSY_BUFS,
           GEO_BUFS, MLP_BUFS, H1_ACT, Y1_ACT, G_ACT, OBY_POOL, IL, PSUM_BF16)
    if key not in _NC_CACHE:
        _NC_CACHE[key] = _build(nbig, repeat, phase)
    return _NC_CACHE[key]


WNAMES = ["W1", "b1", "W2", "b2", "Wd1", "bd1", "Wd2", "bd2", "bias"]


def _run(v, w, wdict, nbig, n_cores, trace=False, repeat=1, phase="full"):
    nc = _get_nc(nbig, repeat, phase)
    rows = nbig * ROWS_BIG
    in_maps = []
    for c in range(n_cores):
        m = {"v": np.ascontiguousarray(v[c * rows:(c + 1) * rows]),
             "w": np.ascontiguousarray(w[c * rows:(c + 1) * rows])}
        for k in WNAMES:
            m[k] = wdict[k]
        in_maps.append(m)
    last_err = None
    for attempt in range(3):
        try:
            res = run_bass_kernel_spmd(nc, in_maps,
                                       core_ids=list(range(n_cores)),
                                       trace=trace)
            break
        except Exception as e:      # transient NRT device errors
            last_err = e
            import time as _t
            _t.sleep(5)
    else:
        raise last_err
    out = np.concatenate([res.results[c]["out"] for c in range(n_cores)], axis=0)
    return out, res


def kernel(**inputs):
    v = np.ascontiguousarray(np.asarray(inputs["v"], dtype=np.float32))
    w = np.ascontiguousarray(np.asarray(inputs["w"], dtype=np.float32))
    wdict = {k: np.ascontiguousarray(np.asarray(inputs[k], dtype=np.float32))
             for k in WNAMES}
    wdict["bias"] = wdict["bias"].reshape(1, 3)
    out, _ = _run(v, w, wdict, BC // ROWS_BIG, NCORES)
    return out
